# revision 1
# baseline (speedup 1.0000x reference)
"""ConvAttention Trainium2 kernel.

Computes, for B=32 batches sharded 4-per-core across 8 NeuronCores:
  keys' = keys + style_emb^T
  k = conv1d_k1(relu(conv1d_k3(keys', kw1, kb1)), kw2, kb2)        [80, 400]
  q = conv1d_k1(relu(conv1d_k1(relu(conv1d_k3(queries, qw1, qb1)), qw2, qb2)), qw3, qb3)
  attn_raw = SCALE * (|q|^2 + |k|^2 - 2 q.k)                        [2000, 400]
  attn_logprob = log_softmax(attn_raw, axis=-1) + log(prior + EPS)
  attn = softmax(where(mask, -inf, attn_logprob), axis=-1)

Key algebraic facts used:
  * per-row (t1) constants cancel in both log_softmax and softmax, so the
    S*|q|^2 term is never computed.
  * the S*|k|^2 term is broadcast across t1 rows by a K=1 matmul with a ones
    stationary vector.
  * softmax(x + log(prior+eps), masked) == E*W / sum(E*W) with
    E = exp(x - rowmax(x)) (already needed for the logsumexp) and
    W = (prior+eps)*maskmul -- no second exponential pass is needed.
"""

import os
import sys
import numpy as np

sys.path.insert(0, "/opt/trn_rl_repo")

B, T1, T2 = 32, 2000, 400
N_MEL, N_TEXT, N_ATT = 80, 512, 80
N_CORES = 8
BPC = B // N_CORES  # batches per core
SCALE = -0.0005
EPS = 1e-8

# t1 chunking for the attention map: 15 x 128 + 80
T1_CHUNKS = [(i * 128, min(128, T1 - i * 128)) for i in range((T1 + 127) // 128)]
# t1 chunking for the q convs (psum free-dim limit 512)
QT_CHUNKS = [(i * 500, 500) for i in range(4)]
# t2 chunking for the style transpose
T2_CHUNKS = [(i * 128, min(128, T2 - i * 128)) for i in range((T2 + 127) // 128)]

_PROGRAM_CACHE = {}


def build_program(mm_f32r=True, skip_tr=False, skip_chunks=False):
    """Builds and compiles the single-core Bass program (SPMD across 8 cores)."""
    import concourse.bass as bass
    import concourse.bacc as bacc
    import concourse.mybir as mybir
    from concourse import tile

    f32 = mybir.dt.float32
    mdt = mybir.dt.float32r if mm_f32r else f32
    AF = mybir.ActivationFunctionType
    ALU = mybir.AluOpType

    nc = bacc.Bacc("TRN2", target_bir_lowering=False, debug=False,
                   num_devices=N_CORES)

    # ---- I/O -------------------------------------------------------------
    queries_h = nc.dram_tensor("queries", [BPC, N_MEL, T1], mdt, kind="ExternalInput").ap()
    keys_h = nc.dram_tensor("keys", [BPC, N_TEXT, T2], f32, kind="ExternalInput").ap()
    style_h = nc.dram_tensor("style", [BPC, T2, N_TEXT], f32, kind="ExternalInput").ap()
    prior_h = nc.dram_tensor("prior", [BPC, T1, T2], f32, kind="ExternalInput").ap()
    maskf_h = nc.dram_tensor("maskf", [BPC, T2], mdt, kind="ExternalInput").ap()
    ident_h = nc.dram_tensor("ident", [128, 128], f32, kind="ExternalInput").ap()

    qw1t_h = nc.dram_tensor("qw1t", [3, N_MEL, 2 * N_MEL], mdt, kind="ExternalInput").ap()
    qb1_h = nc.dram_tensor("qb1", [2 * N_MEL, 1], f32, kind="ExternalInput").ap()
    qw2t_h = nc.dram_tensor("qw2t", [2 * N_MEL, N_MEL], mdt, kind="ExternalInput").ap()
    qb2_h = nc.dram_tensor("qb2", [N_MEL, 1], f32, kind="ExternalInput").ap()
    qw3t_h = nc.dram_tensor("qw3t", [N_MEL, N_ATT], mdt, kind="ExternalInput").ap()
    qb3_h = nc.dram_tensor("qb3", [N_ATT, 1], f32, kind="ExternalInput").ap()
    kw1t_h = nc.dram_tensor("kw1t", [3, N_TEXT, 2 * N_TEXT], mdt, kind="ExternalInput").ap()
    kb1_h = nc.dram_tensor("kb1", [2 * N_TEXT, 1], f32, kind="ExternalInput").ap()
    # kw2ts/kb2s have -2*SCALE folded in on the host
    kw2ts_h = nc.dram_tensor("kw2ts", [2 * N_TEXT, N_ATT], mdt, kind="ExternalInput").ap()
    kb2s_h = nc.dram_tensor("kb2s", [N_ATT, 1], f32, kind="ExternalInput").ap()

    out_attn_h = nc.dram_tensor("out_attn", [BPC, T1, T2], f32, kind="ExternalOutput").ap()
    out_logp_h = nc.dram_tensor("out_logp", [BPC, T1, T2], f32, kind="ExternalOutput").ap()

    with tile.TileContext(nc) as tc:
        from contextlib import ExitStack
        with ExitStack() as ctx:
            const_pool = ctx.enter_context(tc.tile_pool(name="const", bufs=1))
            wpool = ctx.enter_context(tc.tile_pool(name="weights", bufs=1))
            kpath = ctx.enter_context(tc.tile_pool(name="kpath", bufs=2))
            y1pool = ctx.enter_context(tc.tile_pool(name="y1", bufs=10))
            qpath = ctx.enter_context(tc.tile_pool(name="qpath", bufs=2))
            chunkp = ctx.enter_context(tc.tile_pool(name="chunk", bufs=3))
            tmpp = ctx.enter_context(tc.tile_pool(name="tmp", bufs=2))
            smallp = ctx.enter_context(tc.tile_pool(name="small", bufs=4))
            ps_qk = ctx.enter_context(tc.tile_pool(name="ps_qk", bufs=3, space="PSUM"))
            ps_conv = ctx.enter_context(tc.tile_pool(name="ps_conv", bufs=3, space="PSUM"))
            ps_small = ctx.enter_context(tc.tile_pool(name="ps_small", bufs=2, space="PSUM"))

            # ---- constants & weights (loaded once) -----------------------
            ones_f1 = const_pool.tile([1, 128], f32, name="ones_f1")
            nc.vector.memset(ones_f1[:], 1.0)
            ones_f80 = const_pool.tile([N_ATT, 1], f32, name="ones_f80")
            nc.vector.memset(ones_f80[:], 1.0)
            zero_col = const_pool.tile([128, 1], f32, name="zero_col")
            nc.vector.memset(zero_col[:], 0.0)
            if mm_f32r:
                ones1 = const_pool.tile([1, 128], mdt, name="ones1")
                nc.vector.tensor_copy(ones1[:], ones_f1[:])
                ones80 = const_pool.tile([N_ATT, 1], mdt, name="ones80")
                nc.vector.tensor_copy(ones80[:], ones_f80[:])
            else:
                ones1, ones80 = ones_f1, ones_f80
            ident_sb = const_pool.tile([128, 128], f32, name="ident_sb")
            nc.sync.dma_start(out=ident_sb[:], in_=ident_h[:, :])
            eps_col = const_pool.tile([128, 1], f32, name="eps_col")
            nc.vector.memset(eps_col[:], EPS)

            kw1_sb = {}
            for d in range(3):
                for c in range(4):
                    t = wpool.tile([128, 2 * N_TEXT], mdt, name=f"kw1_{d}_{c}")
                    nc.sync.dma_start(out=t[:], in_=kw1t_h[d, 128 * c:128 * (c + 1), :])
                    kw1_sb[(d, c)] = t
            qw1_sb = []
            for d in range(3):
                t = wpool.tile([N_MEL, 2 * N_MEL], mdt, name=f"qw1_{d}")
                nc.sync.dma_start(out=t[:], in_=qw1t_h[d, :, :])
                qw1_sb.append(t)
            qw2_a = wpool.tile([128, N_MEL], mdt, name="qw2_a")
            nc.sync.dma_start(out=qw2_a[:], in_=qw2t_h[0:128, :])
            qw2_b = wpool.tile([32, N_MEL], mdt, name="qw2_b")
            nc.sync.dma_start(out=qw2_b[:], in_=qw2t_h[128:160, :])
            qw3_sb = wpool.tile([N_MEL, N_ATT], mdt, name="qw3_sb")
            nc.sync.dma_start(out=qw3_sb[:], in_=qw3t_h[:, :])
            kw2_sb = []
            for c in range(8):
                t = wpool.tile([128, N_ATT], mdt, name=f"kw2_{c}")
                nc.sync.dma_start(out=t[:], in_=kw2ts_h[128 * c:128 * (c + 1), :])
                kw2_sb.append(t)

            qb1_a = wpool.tile([128, 1], f32, name="qb1_a")
            nc.sync.dma_start(out=qb1_a[:], in_=qb1_h[0:128, :])
            qb1_b = wpool.tile([32, 1], f32, name="qb1_b")
            nc.sync.dma_start(out=qb1_b[:], in_=qb1_h[128:160, :])
            qb2_sb = wpool.tile([N_MEL, 1], f32, name="qb2_sb")
            nc.sync.dma_start(out=qb2_sb[:], in_=qb2_h[:, :])
            qb3_sb = wpool.tile([N_ATT, 1], f32, name="qb3_sb")
            nc.sync.dma_start(out=qb3_sb[:], in_=qb3_h[:, :])
            kb1_sb = []
            for c in range(8):
                t = wpool.tile([128, 1], f32, name=f"kb1_{c}")
                nc.sync.dma_start(out=t[:], in_=kb1_h[128 * c:128 * (c + 1), :])
                kb1_sb.append(t)
            kb2s_sb = wpool.tile([N_ATT, 1], f32, name="kb2s_sb")
            nc.sync.dma_start(out=kb2s_sb[:], in_=kb2s_h[:, :])

            # ---- per-batch work ------------------------------------------
            for b in range(BPC):
                # ---------- key path ----------
                st_sb = []
                for ti, (t0, tw) in enumerate(T2_CHUNKS):
                    t = kpath.tile([tw, N_TEXT], f32, name=f"st_{ti}", tag="st", bufs=6)
                    nc.sync.dma_start(out=t[:], in_=style_h[b, t0:t0 + tw, :])
                    st_sb.append(t)

                ks_sb = []  # keys+style^T, channel-major, zero-padded cols
                for c in range(4):
                    ks_ps = None
                    if not skip_tr:
                      ks_ps = ps_conv.tile([128, T2], f32, name=f"ks_ps_{c}", tag="psc")
                      for ti, (t0, tw) in enumerate(T2_CHUNKS):
                        nc.tensor.transpose(
                            ks_ps[:, t0:t0 + tw],
                            st_sb[ti][:, 128 * c:128 * (c + 1)],
                            ident_sb[0:tw, 0:tw],
                        )
                    kt = tmpp.tile([128, T2], f32, name=f"kt_{c}", tag="kt")
                    nc.sync.dma_start(out=kt[:], in_=keys_h[b, 128 * c:128 * (c + 1), :])
                    ks = kpath.tile([128, T2 + 2], mdt, name=f"ks_{c}", tag="ks", bufs=8)
                    nc.vector.tensor_copy(ks[:, 0:1], zero_col[:])
                    nc.vector.tensor_copy(ks[:, T2 + 1:T2 + 2], zero_col[:])
                    if skip_tr:
                        nc.vector.tensor_copy(ks[:, 1:T2 + 1], kt[:])
                    else:
                        nc.vector.tensor_add(ks[:, 1:T2 + 1], kt[:], ks_ps[:])
                    ks_sb.append(ks)

                # conv1 (k3, 512 -> 1024) + relu
                y1_sb = []
                for j in range(8):
                    c1 = ps_conv.tile([128, T2], f32, name=f"c1_{j}", tag="psc")
                    n = 0
                    for c in range(4):
                        for d in range(3):
                            nc.tensor.matmul(
                                c1[:],
                                kw1_sb[(d, c)][:, 128 * j:128 * (j + 1)],
                                ks_sb[c][:, d:d + T2],
                                start=(n == 0), stop=(n == 11),
                            )
                            n += 1
                    y1 = y1pool.tile([128, T2], mdt, name=f"y1_{j}", tag="y1")
                    nc.scalar.activation(y1[:], c1[:], AF.Relu, bias=kb1_sb[j][:])
                    y1_sb.append(y1)

                # conv2 (k1, 1024 -> 80), -2*SCALE folded in
                k_ps = ps_conv.tile([N_ATT, T2], f32, name="k_ps", tag="psc")
                for c in range(8):
                    nc.tensor.matmul(k_ps[:], kw2_sb[c][:], y1_sb[c][:],
                                     start=(c == 0), stop=(c == 7))
                b0 = kpath.tile([N_ATT, T2], mdt, name="b0", tag="b0")
                nc.scalar.activation(b0[:], k_ps[:], AF.Identity, bias=kb2s_sb[:])

                # S*|k|^2 row:  sum(b0^2) / (4*SCALE)
                ksq = tmpp.tile([N_ATT, T2], mdt, name="ksq", tag="ksq")
                nc.vector.tensor_mul(ksq[:], b0[:], b0[:])
                k2_ps = ps_small.tile([1, T2], f32, name="k2_ps", tag="pss")
                nc.tensor.matmul(k2_ps[:], ones80[:], ksq[:], start=True, stop=True)
                bk2 = kpath.tile([1, T2], mdt, name="bk2", tag="bk2")
                nc.scalar.activation(bk2[:], k2_ps[:], AF.Copy, scale=1.0 / (4.0 * SCALE))

                # mask row -> broadcast to [128, T2]
                mrow = smallp.tile([1, T2], mdt, name="mrow", tag="mrow")
                nc.sync.dma_start(out=mrow[0:1, :], in_=maskf_h[b:b + 1, :])
                mb_ps = ps_small.tile([128, T2], f32, name="mb_ps", tag="pss")
                nc.tensor.matmul(mb_ps[:], ones1[:], mrow[:], start=True, stop=True)
                mmul = kpath.tile([128, T2], f32, name="mmul", tag="mmul")
                nc.vector.tensor_copy(mmul[:], mb_ps[:])

                # ---------- query path ----------
                q_in = qpath.tile([N_MEL, T1 + 2], mdt, name="q_in", tag="q_in")
                nc.vector.tensor_copy(q_in[:, 0:1], zero_col[0:N_MEL, :])
                nc.vector.tensor_copy(q_in[:, T1 + 1:T1 + 2], zero_col[0:N_MEL, :])
                nc.sync.dma_start(out=q_in[:, 1:T1 + 1], in_=queries_h[b, :, :])

                q_fin = qpath.tile([N_ATT, T1], mdt, name="q_fin", tag="q_fin")
                for (t0, tw) in QT_CHUNKS:
                    y1qa = tmpp.tile([128, tw], mdt, name=f"y1qa_{t0}", tag="y1qa", bufs=3)
                    y1qb = tmpp.tile([32, tw], mdt, name=f"y1qb_{t0}", tag="y1qb", bufs=3)
                    for (p0, p1, bt, yt) in (
                        (0, 128, qb1_a, y1qa),
                        (128, 160, qb1_b, y1qb),
                    ):
                        pw = p1 - p0
                        q1 = ps_conv.tile([pw, tw], f32, name=f"q1_{t0}_{p0}", tag="psc")
                        for d in range(3):
                            nc.tensor.matmul(q1[:], qw1_sb[d][:, p0:p1],
                                             q_in[:, d + t0:d + t0 + tw],
                                             start=(d == 0), stop=(d == 2))
                        nc.scalar.activation(yt[:], q1[:], AF.Relu, bias=bt[:])

                    q2 = ps_conv.tile([N_MEL, tw], f32, name=f"q2_{t0}", tag="psc")
                    nc.tensor.matmul(q2[:], qw2_a[:], y1qa[:],
                                     start=True, stop=False)
                    nc.tensor.matmul(q2[:], qw2_b[:], y1qb[:],
                                     start=False, stop=True)
                    q_mid = tmpp.tile([N_MEL, tw], mdt, name=f"q_mid_{t0}", tag="q_mid", bufs=3)
                    nc.scalar.activation(q_mid[:], q2[:], AF.Relu, bias=qb2_sb[:])

                    q3 = ps_conv.tile([N_ATT, tw], f32, name=f"q3_{t0}", tag="psc")
                    nc.tensor.matmul(q3[:], qw3_sb[:], q_mid[:],
                                     start=True, stop=True)
                    nc.scalar.activation(q_fin[:, t0:t0 + tw], q3[:], AF.Identity, bias=qb3_sb[:])

                # ---------- attention chunks ----------
                if skip_chunks:
                    nc.sync.dma_start(out=out_attn_h[b, 0:N_ATT, :], in_=b0[:])
                    nc.sync.dma_start(out=out_logp_h[b, 0:N_ATT, :], in_=q_fin[:, 0:T2])
                    continue
                for ci, (r0, w) in enumerate(T1_CHUNKS):
                    pr = chunkp.tile([w, T2], f32, name=f"pr_{ci}", tag="pr", bufs=4)
                    nc.sync.dma_start(out=pr[:], in_=prior_h[b, r0:r0 + w, :])
                    lp = chunkp.tile([w, T2], f32, name=f"lp_{ci}", tag="lp", bufs=4)
                    nc.scalar.activation(lp[:], pr[:], AF.Ln, bias=eps_col[0:w, :])
                    # W = prior * maskmul (the +eps term is ~1e-8 relative: negligible)
                    wt_ = chunkp.tile([w, T2], f32, name=f"wt_{ci}", tag="wt", bufs=3)
                    nc.vector.tensor_mul(wt_[:], pr[:], mmul[0:w, :])

                    p0 = ps_qk.tile([w, T2], f32, name=f"p0_{ci}", tag="psqk")
                    nc.tensor.matmul(p0[:], q_fin[:, r0:r0 + w], b0[:],
                                     start=True, stop=False)
                    nc.tensor.matmul(p0[:], ones1[:, 0:w], bk2[:],
                                     start=False, stop=True)

                    # logits are bounded (|attn_raw - S*q2| <~ 1), so exp needs
                    # no max-shift; accum_out gives the row sum in the same pass
                    e = chunkp.tile([w, T2], f32, name=f"e_{ci}", tag="e", bufs=4)
                    rsum = smallp.tile([w, 1], f32, name=f"rsum_{ci}", tag="rsum")
                    nc.scalar.activation(e[:], p0[:], AF.Exp, accum_out=rsum[:])
                    lnr = smallp.tile([w, 1], f32, name=f"lnr_{ci}", tag="lnr")
                    nc.scalar.activation(lnr[:], rsum[:], AF.Ln)
                    cc = smallp.tile([w, 1], f32, name=f"cc_{ci}", tag="cc")
                    nc.vector.tensor_scalar_mul(cc[:], lnr[:], -1.0)

                    # attn_logprob = (p0 + cc) + lp
                    t1 = chunkp.tile([w, T2], f32, name=f"t1_{ci}", tag="t1", bufs=2)
                    nc.scalar.activation(t1[:], p0[:], AF.Identity, bias=cc[:])
                    o1 = chunkp.tile([w, T2], f32, name=f"o1_{ci}", tag="o1", bufs=2)
                    nc.vector.tensor_add(o1[:], t1[:], lp[:])
                    nc.sync.dma_start(out=out_logp_h[b, r0:r0 + w, :], in_=o1[:])

                    # attn = e*W / sum(e*W)
                    u = chunkp.tile([w, T2], f32, name=f"u_{ci}", tag="u", bufs=4)
                    nc.vector.tensor_mul(u[:], e[:], wt_[:])
                    rsum2 = smallp.tile([w, 1], f32, name=f"rsum2_{ci}", tag="rsum2")
                    nc.vector.tensor_reduce(rsum2[:], u[:], axis=mybir.AxisListType.X,
                                            op=ALU.add)
                    rrec = smallp.tile([w, 1], f32, name=f"rrec_{ci}", tag="rrec")
                    nc.vector.reciprocal(rrec[:], rsum2[:])
                    o2 = chunkp.tile([w, T2], f32, name=f"o2_{ci}", tag="o2", bufs=2)
                    nc.vector.tensor_scalar_mul(o2[:], u[:], rrec[:])
                    nc.sync.dma_start(out=out_attn_h[b, r0:r0 + w, :], in_=o2[:])

    nc.compile()
    return nc


def get_program(mm_f32r=True):
    key = ("prog", mm_f32r)
    if key not in _PROGRAM_CACHE:
        _PROGRAM_CACHE[key] = build_program(mm_f32r)
    return _PROGRAM_CACHE[key]


def make_in_maps(inputs):
    """Host-side prep: shard per core, transpose/fold weights."""
    queries = np.asarray(inputs["queries"], np.float32)
    keys = np.asarray(inputs["keys"], np.float32)
    mask = np.asarray(inputs["mask"])
    prior = np.asarray(inputs["attn_prior"], np.float32)
    style = np.asarray(inputs["style_emb"], np.float32)

    qw1 = np.asarray(inputs["qw1"], np.float32)
    qb1 = np.asarray(inputs["qb1"], np.float32)
    qw2 = np.asarray(inputs["qw2"], np.float32)
    qb2 = np.asarray(inputs["qb2"], np.float32)
    qw3 = np.asarray(inputs["qw3"], np.float32)
    qb3 = np.asarray(inputs["qb3"], np.float32)
    kw1 = np.asarray(inputs["kw1"], np.float32)
    kb1 = np.asarray(inputs["kb1"], np.float32)
    kw2 = np.asarray(inputs["kw2"], np.float32)
    kb2 = np.asarray(inputs["kb2"], np.float32)

    fold = -2.0 * SCALE
    qw1t = np.ascontiguousarray(qw1.transpose(2, 1, 0))        # [3, 80, 160]
    qw2t = np.ascontiguousarray(qw2[:, :, 0].T)                # [160, 80]
    qw3t = np.ascontiguousarray(qw3[:, :, 0].T)                # [80, 80]
    kw1t = np.ascontiguousarray(kw1.transpose(2, 1, 0))        # [3, 512, 1024]
    kw2ts = np.ascontiguousarray(kw2[:, :, 0].T * fold)        # [1024, 80]
    kb2s = (kb2 * fold).reshape(-1, 1).astype(np.float32)

    maskf = np.where(mask[:, :, 0], 0.0, 1.0).astype(np.float32)  # [B, 400]
    ident = np.eye(128, dtype=np.float32)

    shared = dict(
        ident=ident,
        qw1t=qw1t, qb1=qb1.reshape(-1, 1).astype(np.float32),
        qw2t=qw2t, qb2=qb2.reshape(-1, 1).astype(np.float32),
        qw3t=qw3t, qb3=qb3.reshape(-1, 1).astype(np.float32),
        kw1t=kw1t, kb1=kb1.reshape(-1, 1).astype(np.float32),
        kw2ts=kw2ts, kb2s=kb2s,
    )
    in_maps = []
    for c in range(N_CORES):
        sl = slice(c * BPC, (c + 1) * BPC)
        m = dict(shared)
        m["queries"] = np.ascontiguousarray(queries[sl])
        m["keys"] = np.ascontiguousarray(keys[sl])
        m["style"] = np.ascontiguousarray(style[sl])
        m["prior"] = np.ascontiguousarray(prior[sl])
        m["maskf"] = np.ascontiguousarray(maskf[sl])
        in_maps.append(m)
    return in_maps


def kernel(**inputs):
    from concourse.bass_utils import run_bass_kernel_spmd

    nc = get_program(os.environ.get("MM_F32R", "1") == "1")
    in_maps = make_in_maps(inputs)
    res = run_bass_kernel_spmd(nc, in_maps, list(range(N_CORES)))
    attn = np.concatenate([r["out_attn"] for r in res.results], axis=0)
    logp = np.concatenate([r["out_logp"] for r in res.results], axis=0)
    attn = attn.reshape(B, 1, T1, T2)
    logp = logp.reshape(B, 1, T1, T2)
    return attn, logp

